# revision 6
# baseline (speedup 1.0000x reference)
"""Trainium2 Bass kernel for nn_CrossAttention (8-core data-parallel over batch).

Math (per batch b):
  x1 = x + PEx ; y1 = y + PEy           (raw-reshape positional encodings)
  q  = conv3x3(relu(conv3x3(x1,wq1)+bq1), wq2)+bq2   viewed as (1024,128)
  k  = conv3x3(relu(conv3x3(y1,wk1)+bk1), wk2)+bk2   viewed as (4096,128)
  out = softmax(s * q @ k.T) @ z.flat                (s = 1/sqrt(128))

Device mapping (one batch element per NeuronCore):
  - x+PE folded host-side; all matmul operands fp16 (1 cyc/col on PE vs
    fp32r's LOW_HIGH two-pass) with fp32 PSUM accumulation
  - convs as 9 accumulating fp16 matmuls per 512-wide output tile (weights
    stationary (ci,co), padded image moving with 2D shifted APs)
  - conv bias(+relu) applied on DVE (tensor_scalar with per-partition scalar
    AP), freeing ScalarE for the softmax exps
  - PE transposes conv outputs into j-major layout for the attention
    contraction; qT pre-scaled by s during the PSUM->SBUF copy
  - logits in PSUM; softmax shift = row max over the first 256 psl columns
    (covers two full channel sweeps; worst observed gap to true chunk max is
    ~33 << 88 so exp stays in fp32/bf16 range); online rescaling combines the
    4 key-chunks
  - exp on ScalarE with fused per-partition bias + accumulated denominator;
    numerator via DVE scalar_tensor_tensor (P*v with fused sum), deferred one
    m-iteration so ScalarE->DVE->ScalarE serialization never forms
  - k-path conv/transpose work is sliced into single-instruction thunks and
    spread uniformly across the attention m-iterations of chunks 0-2 so the
    PE stays dense (and HAM-warm) while ScalarE chews the exps
"""

import numpy as np
import ml_dtypes

import concourse.bass as bass
import concourse.mybir as mybir
import concourse.tile as tile
from concourse import bacc
from concourse.bass import ts
from concourse.bass_utils import run_bass_kernel_spmd

F32 = mybir.dt.float32
F16 = mybir.dt.float16
BF16 = mybir.dt.bfloat16
AF = mybir.ActivationFunctionType
ALU = mybir.AluOpType

C = 128
A = 32          # q spatial side
H = 64          # k spatial side
SQ = A * A      # 1024
SK = H * H      # 4096
SCALE = float(C ** -0.5)
N_CORES = 8


def _make_pe(dim, length):
    pos = np.arange(length, dtype=np.float32)[:, None]
    div = np.exp(np.arange(0, dim, 2, dtype=np.float32) * np.float32(-np.log(10000.0) / dim))
    pe = np.zeros((length, dim), dtype=np.float32)
    pe[:, 0::2] = np.sin(pos * div)
    pe[:, 1::2] = np.cos(pos * div)
    return pe


def _build_program():
    nc = bacc.Bacc("TRN2", target_bir_lowering=False, debug=False, num_devices=N_CORES)

    dx = nc.dram_tensor("x1", [C, SQ], F16, kind="ExternalInput")
    dy = nc.dram_tensor("y1", [C, SK], F16, kind="ExternalInput")
    dv = nc.dram_tensor("vz", [1, SK], BF16, kind="ExternalInput")
    dw = {n: nc.dram_tensor(n, [C, 9 * C], F16, kind="ExternalInput")
          for n in ("wq1", "wq2", "wk1", "wk2")}
    db = {n: nc.dram_tensor(n, [C, 1], F32, kind="ExternalInput")
          for n in ("bq1", "bq2", "bk1", "bk2")}
    dident = nc.dram_tensor("ident", [C, C], F16, kind="ExternalInput")
    dout = nc.dram_tensor("out", [SQ, 1], F32, kind="ExternalOutput")

    XP, YP = A + 2, H + 2          # padded sides: 34, 66
    with tile.TileContext(nc) as tc:
        with (
            tc.tile_pool(name="const", bufs=1) as cst,
            tc.tile_pool(name="kimg", bufs=2) as kip,
            tc.tile_pool(name="pp", bufs=3) as ppool,
            tc.tile_pool(name="scr", bufs=2) as scrp,
            tc.tile_pool(name="acc", bufs=2) as accp,
            tc.tile_pool(name="psc", bufs=2, space="PSUM") as psc,
            tc.tile_pool(name="pst", bufs=2, space="PSUM") as pst,
            tc.tile_pool(name="psa", bufs=2, space="PSUM") as psa,
        ):
            # ---- constants / inputs to SBUF (q-critical first) ----
            w_sb, b_sb = {}, {}

            def load_w(n):
                w_sb[n] = cst.tile([C, 9 * C], F16, tag=n, name=n + "_sb")
                nc.sync.dma_start(out=w_sb[n][:, 0:576], in_=dw[n].ap()[:, 0:576])
                nc.sync.dma_start(out=w_sb[n][:, 576:1152], in_=dw[n].ap()[:, 576:1152])

            def load_b(n):
                b_sb[n] = cst.tile([C, 1], F32, tag=n, name=n + "_sb")
                nc.sync.dma_start(out=b_sb[n][:], in_=db[n].ap())

            # padded conv input/intermediate buffers (fp16)
            x_pad = cst.tile([C, XP * XP], F16, tag="x_pad")
            y_pad = cst.tile([C, YP * YP], F16, tag="y_pad")
            t1q = cst.tile([C, XP * XP], F16, tag="t1q")
            t1k = cst.tile([C, YP * YP], F16, tag="t1k")
            x_pad3 = x_pad[:].rearrange("p (r c) -> p r c", c=XP)
            y_pad3 = y_pad[:].rearrange("p (r c) -> p r c", c=YP)
            t1q3 = t1q[:].rearrange("p (r c) -> p r c", c=XP)
            t1k3 = t1k[:].rearrange("p (r c) -> p r c", c=YP)

            load_w("wq1"); load_b("bq1")
            nc.sync.dma_start(out=x_pad3[:, 1:A + 1, 1:A + 1],
                              in_=dx.ap().rearrange("p (r c) -> p r c", c=A))
            load_w("wq2"); load_b("bq2")
            ident = cst.tile([C, C], F16, tag="ident")
            nc.sync.dma_start(out=ident[:], in_=dident.ap())
            for h in range(2):
                nc.sync.dma_start(
                    out=y_pad3[:, 1 + (H // 2) * h: 1 + (H // 2) * (h + 1), 1:H + 1],
                    in_=dy.ap().rearrange("p (r c) -> p r c", c=H)[:, ts(h, H // 2), :])
            load_w("wk1"); load_b("bk1"); load_w("wk2"); load_b("bk2")
            v_rep = cst.tile([C, SK], BF16, tag="v_rep")
            for h in range(2):
                nc.sync.dma_start(out=v_rep[:, ts(h, SK // 2)],
                                  in_=dv.ap()[:, ts(h, SK // 2)].broadcast_to((C, SK // 2)))

            # ---- zero borders of padded buffers ----
            zrow = cst.tile([C, YP], F16, tag="zrow")
            nc.vector.memset(zrow[:], 0.0)

            def pad_borders(t3, side):
                zr = zrow[:, 0:side].rearrange("p (a c) -> p a c", a=1)
                zc = zrow[:, 0:side - 2].rearrange("p (r a) -> p r a", a=1)
                nc.vector.tensor_copy(t3[:, 0:1, :], zr)
                nc.vector.tensor_copy(t3[:, side - 1:side, :], zr)
                nc.vector.tensor_copy(t3[:, 1:side - 1, 0:1], zc)
                nc.vector.tensor_copy(t3[:, 1:side - 1, side - 1:side], zc)

            pad_borders(x_pad3, XP)
            pad_borders(t1q3, XP)
            pad_borders(y_pad3, YP)
            pad_borders(t1k3, YP)

            # warm the ScalarE exp table before the attention phase needs it
            dummy = cst.tile([C, 1], F32, tag="dummy")
            nc.scalar.activation(dummy[:], b_sb["bq1"][:], AF.Exp, scale=0.0)

            q_img = cst.tile([C, SQ], F16, tag="q_img")
            qT = cst.tile([C, SQ], F16, tag="qT")
            kT = cst.tile([C, SK], F16, tag="kT")
            negM = cst.tile([C, 8], F32, tag="negM")
            denom = cst.tile([C, 8], F32, tag="denom")
            numer = cst.tile([C, 8], F32, tag="numer")

            def conv_mm(ps3, w, i, src3, rows0, nrows, side_c):
                dyy, dxx = i // 3, i % 3
                rhs = src3[:, rows0 + dyy: rows0 + dyy + nrows, dxx: dxx + side_c]
                nc.tensor.matmul(ps3, w[:, ts(i, C)], rhs,
                                 start=(i == 0), stop=(i == 8))

            # ---- q path (acts on DVE; ScalarE reserved for exp) ----
            for n in range(2):   # conv1: tiles of 16 rows x 32 cols = 512
                ps = psc.tile([C, 512], F32, tag="cps")
                ps3 = ps[:].rearrange("p (r c) -> p r c", c=A)
                for i in range(9):
                    conv_mm(ps3, w_sb["wq1"], i, x_pad3, 16 * n, 16, A)
                nc.vector.tensor_scalar(
                    out=t1q3[:, 16 * n + 1:16 * n + 17, 1:A + 1], in0=ps3,
                    scalar1=b_sb["bq1"][:], scalar2=0.0, op0=ALU.add, op1=ALU.max)
            for n in range(2):   # conv2 + bias
                ps = psc.tile([C, 512], F32, tag="cps")
                ps3 = ps[:].rearrange("p (r c) -> p r c", c=A)
                for i in range(9):
                    conv_mm(ps3, w_sb["wq2"], i, t1q3, 16 * n, 16, A)
                nc.vector.tensor_scalar(
                    out=q_img[:, ts(n, 512)], in0=ps[:],
                    scalar1=b_sb["bq2"][:], scalar2=None, op0=ALU.add)
            for g in range(2):   # transpose groups of 4 t-blocks; qT pre-scaled by s
                pt = pst.tile([C, 512], F16, tag="tps")
                for i in range(4):
                    nc.tensor.transpose(pt[:, ts(i, C)],
                                        q_img[:, ts(4 * g + i, C)], ident[:])
                nc.vector.tensor_scalar(out=qT[:, ts(g, 512)], in0=pt[:],
                                        scalar1=SCALE, scalar2=None, op0=ALU.mult)

            # ---- k path, sliced into single-instruction thunks ----
            def conv1_k_thunks(t):
                box = {}
                th = []

                def mk(i):
                    def f():
                        if i == 0:
                            box['ps'] = psc.tile([C, 512], F32, tag="cps", name="cps")
                        ps3 = box['ps'][:].rearrange("p (r c) -> p r c", c=H)
                        conv_mm(ps3, w_sb["wk1"], i, y_pad3, 8 * t, 8, H)
                    return f
                for i in range(9):
                    th.append(mk(i))

                def act():
                    ps3 = box['ps'][:].rearrange("p (r c) -> p r c", c=H)
                    nc.vector.tensor_scalar(
                        out=t1k3[:, 8 * t + 1:8 * t + 9, 1:H + 1], in0=ps3,
                        scalar1=b_sb["bk1"][:], scalar2=0.0, op0=ALU.add, op1=ALU.max)
                th.append(act)
                return th

            def conv2_k_thunks(t):
                box = {}
                th = []

                def mk(i):
                    def f():
                        if i == 0:
                            box['ps'] = psc.tile([C, 512], F32, tag="cps", name="cps")
                        ps3 = box['ps'][:].rearrange("p (r c) -> p r c", c=H)
                        conv_mm(ps3, w_sb["wk2"], i, t1k3, 8 * t, 8, H)
                    return f
                for i in range(9):
                    th.append(mk(i))

                def act():
                    box['kimg'] = kip.tile([C, 512], F16, tag="kimg", name="kimg")
                    nc.vector.tensor_scalar(
                        out=box['kimg'][:], in0=box['ps'][:],
                        scalar1=b_sb["bk2"][:], scalar2=None, op0=ALU.add)
                th.append(act)

                def mk_tr(i):
                    def f():
                        if i == 0:
                            box['pt'] = pst.tile([C, 512], F16, tag="tps", name="tps")
                        nc.tensor.transpose(box['pt'][:, ts(i, C)],
                                            box['kimg'][:, ts(i, C)], ident[:])
                    return f
                for i in range(4):
                    th.append(mk_tr(i))

                def cp():
                    nc.vector.tensor_copy(kT[:, ts(t, 512)], box['pt'][:])
                th.append(cp)
                return th

            # prologue + chunk-0 pre-work: kT chunk 0 (tiles 0,1).
            # Ordering invariant: conv2_k(t) needs conv1_k(t) AND conv1_k(t+1)
            # (its bottom halo row), so conv1 stays two tiles ahead.
            for f in conv1_k_thunks(0) + conv1_k_thunks(1) \
                    + conv1_k_thunks(2) + conv1_k_thunks(3) \
                    + conv2_k_thunks(0) + conv2_k_thunks(1):
                f()

            def pre_work(cn):
                # work producing kT chunk cn (interleaved into chunk cn-1's m-loop)
                if cn > 3:
                    return []
                th = []
                if 2 * cn + 2 < 8:
                    th += conv1_k_thunks(2 * cn + 2)
                if 2 * cn + 3 < 8:
                    th += conv1_k_thunks(2 * cn + 3)
                th += conv2_k_thunks(2 * cn)
                th += conv2_k_thunks(2 * cn + 1)
                return th

            # ---- attention: 4 chunks of 1024 keys, 8 m-blocks each ----
            pend_stt = None    # deferred numerator STT (breaks Sc->DVE->Sc chain)
            for c in range(4):
                work = pre_work(c + 1)
                wpos = 0
                negmax_c = accp.tile([C, 8], F32, tag="negmax_c")
                d_c = accp.tile([C, 8], F32, tag="d_c")
                n_c = accp.tile([C, 8], F32, tag="n_c")
                for m in range(8):
                    psl = psa.tile([C, 1024], F32, tag="psl")
                    for u in range(2):
                        nc.tensor.matmul(psl[:, ts(u, 512)], qT[:, ts(m, C)],
                                         kT[:, 1024 * c + 512 * u: 1024 * c + 512 * (u + 1)],
                                         start=True, stop=True)
                    # shift: minus the max over the first two t-blocks
                    nc.vector.tensor_reduce(out=negmax_c[:, m:m + 1], in_=psl[:, 0:256],
                                            axis=mybir.AxisListType.X, op=ALU.max,
                                            negate=True)
                    P = ppool.tile([C, 1024], BF16, tag="P")
                    nc.scalar.activation(P[:], psl[:], AF.Exp, bias=negmax_c[:, m:m + 1],
                                         scale=1.0, accum_out=d_c[:, m:m + 1])
                    # spread next-chunk conv thunks uniformly over the 8 m-iters
                    tgt = ((m + 1) * len(work) + 7) // 8
                    while wpos < min(tgt, len(work)):
                        work[wpos]()
                        wpos += 1
                    if pend_stt is not None:
                        pend_stt()

                    def mk_stt(P=P, c=c, n_c=n_c, m=m):
                        def f():
                            scrap = scrp.tile([C, 1024], BF16, tag="scrap")
                            nc.vector.scalar_tensor_tensor(
                                out=scrap[:], in0=P[:], scalar=1.0,
                                in1=v_rep[:, ts(c, 1024)],
                                op0=ALU.bypass, op1=ALU.mult,
                                accum_out=n_c[:, m:m + 1])
                        return f
                    pend_stt = mk_stt()
                pend_stt()
                pend_stt = None

                if c == 0:
                    nc.vector.tensor_copy(negM[:], negmax_c[:])
                    nc.vector.tensor_copy(denom[:], d_c[:])
                    nc.vector.tensor_copy(numer[:], n_c[:])
                else:
                    nmin = scrp.tile([C, 8], F32, tag="nmin")
                    diffs = scrp.tile([C, 16], F32, tag="diffs")
                    alphas = scrp.tile([C, 16], F32, tag="alphas")
                    t8 = scrp.tile([C, 8], F32, tag="t8")
                    nc.vector.tensor_tensor(out=nmin[:], in0=negM[:], in1=negmax_c[:], op=ALU.min)
                    nc.vector.tensor_tensor(out=diffs[:, 0:8], in0=nmin[:], in1=negM[:], op=ALU.subtract)
                    nc.vector.tensor_tensor(out=diffs[:, 8:16], in0=nmin[:], in1=negmax_c[:], op=ALU.subtract)
                    nc.scalar.activation(alphas[:], diffs[:], AF.Exp, scale=1.0)
                    nc.vector.tensor_tensor(out=denom[:], in0=denom[:], in1=alphas[:, 0:8], op=ALU.mult)
                    nc.vector.tensor_tensor(out=t8[:], in0=d_c[:], in1=alphas[:, 8:16], op=ALU.mult)
                    nc.vector.tensor_tensor(out=denom[:], in0=denom[:], in1=t8[:], op=ALU.add)
                    nc.vector.tensor_tensor(out=numer[:], in0=numer[:], in1=alphas[:, 0:8], op=ALU.mult)
                    nc.vector.tensor_tensor(out=t8[:], in0=n_c[:], in1=alphas[:, 8:16], op=ALU.mult)
                    nc.vector.tensor_tensor(out=numer[:], in0=numer[:], in1=t8[:], op=ALU.add)
                    nc.vector.tensor_copy(negM[:], nmin[:])

            recip = cst.tile([C, 8], F32, tag="recip")
            res = cst.tile([C, 8], F32, tag="res")
            nc.vector.reciprocal(recip[:], denom[:])
            nc.vector.tensor_tensor(out=res[:], in0=numer[:], in1=recip[:], op=ALU.mult)
            nc.sync.dma_start(out=dout.ap().rearrange("(co m) one -> co (m one)", m=8),
                              in_=res[:])

    nc.compile()
    return nc


_NC_CACHE = []


def _build_in_maps(x, y, z, wq1, bq1, wq2, bq2, wk1, bk1, wk2, bk2):
    x = np.asarray(x, dtype=np.float32)
    y = np.asarray(y, dtype=np.float32)
    z = np.asarray(z, dtype=np.float32)
    B = x.shape[0]

    # weights: (co, ci, dy, dx) -> (ci, tap*128+co), fp16
    wmap = {}
    for name, w in (("wq1", wq1), ("wq2", wq2), ("wk1", wk1), ("wk2", wk2)):
        wmap[name] = np.ascontiguousarray(
            np.asarray(w, dtype=np.float32).transpose(1, 2, 3, 0).reshape(C, 9 * C)
        ).astype(np.float16)
    bmap = {"bq1": bq1, "bq2": bq2, "bk1": bk1, "bk2": bk2}
    bmap = {n: np.ascontiguousarray(np.asarray(b, dtype=np.float32).reshape(C, 1))
            for n, b in bmap.items()}
    pex = _make_pe(C, SQ).reshape(C, SQ)
    pey = _make_pe(C, SK).reshape(C, SK)
    ident = np.eye(C, dtype=np.float16)
    # v in t-major key order: store[t*128+co] = z_flat[co*32+t]
    zperm = np.ascontiguousarray(
        z.reshape(B, SK).reshape(B, C, SK // C).transpose(0, 2, 1).reshape(B, 1, SK)
    ).astype(ml_dtypes.bfloat16)

    in_maps = []
    for b in range(B):
        m = {
            "x1": (x[b].reshape(C, SQ) + pex).astype(np.float16),
            "y1": (y[b].reshape(C, SK) + pey).astype(np.float16),
            "vz": zperm[b],
            "ident": ident,
        }
        m.update(wmap)
        m.update(bmap)
        in_maps.append(m)
    return in_maps


def kernel(x, y, z, wq1, bq1, wq2, bq2, wk1, bk1, wk2, bk2):
    B = np.asarray(x).shape[0]
    assert B == N_CORES

    if not _NC_CACHE:
        _NC_CACHE.append(_build_program())
    nc = _NC_CACHE[0]

    in_maps = _build_in_maps(x, y, z, wq1, bq1, wq2, bq2, wk1, bk1, wk2, bk2)
    res = run_bass_kernel_spmd(nc, in_maps, core_ids=list(range(N_CORES)))
    out = np.stack([res.results[b]["out"].reshape(SQ, 1) for b in range(B)])
    return out.astype(np.float32)


# revision 8
# speedup vs baseline: 1.1700x; 1.1700x over previous
"""Trainium2 Bass kernel for nn_CrossAttention (8-core data-parallel over batch).

Math (per batch b):
  x1 = x + PEx ; y1 = y + PEy           (raw-reshape positional encodings)
  q  = conv3x3(relu(conv3x3(x1,wq1)+bq1), wq2)+bq2   viewed as (1024,128)
  k  = conv3x3(relu(conv3x3(y1,wk1)+bk1), wk2)+bk2   viewed as (4096,128)
  out = softmax(s * q @ k.T) @ z.flat                (s = 1/sqrt(128))

Device mapping (one batch element per NeuronCore):
  - x+PE folded host-side; all matmul operands BF16 (the only full-rate PE
    dtype on TRN2 silicon: fp32r and fp16 both run as two passes) with fp32
    PSUM accumulation
  - convs as 9 accumulating bf16 matmuls per 512-wide output tile (weights
    stationary (ci,co), padded image moving with 2D shifted APs)
  - q-path conv acts on ScalarE (idle early; SCALE folded into the conv2
    activation's free affine), k-path conv acts on DVE (tensor_scalar with
    per-partition bias AP + relu via the second scalar op)
  - conv outputs transposed into j-major layout via DMA XBAR transposes
    (128x128 bf16 blocks issued on SP) - no PE transpose columns, no PSUM
    transpose pool, no DVE fixup copies
  - v replicated across partitions with a GpSimd partition_broadcast; y-pad
    interior filled by GpSimd copies from a contiguous staging tile
  - softmax shift: ONE per-row shift from chunk 0's first-256-column
    subsample (worst observed full-row gap is 36 << 88, so exp stays in
    fp32/bf16 range and no online rescaling is needed; partial num/den sums
    accumulate additively across the 4 key-chunks)
  - exp on ScalarE with fused per-partition bias + accumulated denominator
    (the compiler forbids non-matmul BF16 PSUM writes, so P lands in SBUF);
    numerator via DVE
    scalar_tensor_tensor (P*v with fused sum), deferred one m-iteration so
    ScalarE->DVE->ScalarE serialization never forms
  - k-path conv work is sliced into single-instruction thunks and spread
    uniformly across the attention m-iterations so the PE stays dense (and
    HAM-warm) while ScalarE chews the exps
"""

import numpy as np
import ml_dtypes

import concourse.bass as bass
import concourse.mybir as mybir
import concourse.tile as tile
from concourse import bacc
from concourse.bass import ts
from concourse.bass_utils import run_bass_kernel_spmd

F32 = mybir.dt.float32
BF16 = mybir.dt.bfloat16
AF = mybir.ActivationFunctionType
ALU = mybir.AluOpType

C = 128
A = 32          # q spatial side
H = 64          # k spatial side
SQ = A * A      # 1024
SK = H * H      # 4096
SCALE = float(C ** -0.5)
N_CORES = 8


def _make_pe(dim, length):
    pos = np.arange(length, dtype=np.float32)[:, None]
    div = np.exp(np.arange(0, dim, 2, dtype=np.float32) * np.float32(-np.log(10000.0) / dim))
    pe = np.zeros((length, dim), dtype=np.float32)
    pe[:, 0::2] = np.sin(pos * div)
    pe[:, 1::2] = np.cos(pos * div)
    return pe


def _build_program():
    nc = bacc.Bacc("TRN2", target_bir_lowering=False, debug=False, num_devices=N_CORES)

    dx = nc.dram_tensor("x1", [C, SQ], BF16, kind="ExternalInput")
    dy = nc.dram_tensor("y1", [C, SK], BF16, kind="ExternalInput")
    dv = nc.dram_tensor("vz", [1, SK], BF16, kind="ExternalInput")
    dw = {n: nc.dram_tensor(n, [C, 9 * C], BF16, kind="ExternalInput")
          for n in ("wq1", "wq2", "wk1", "wk2")}
    db = {n: nc.dram_tensor(n, [C, 1], F32, kind="ExternalInput")
          for n in ("bq1", "bq2", "bk1", "bk2")}
    dout = nc.dram_tensor("out", [SQ, 1], F32, kind="ExternalOutput")

    XP, YP = A + 2, H + 2          # padded sides: 34, 66
    with tile.TileContext(nc) as tc:
        with (
            tc.tile_pool(name="const", bufs=1) as cst,
            tc.tile_pool(name="kimg", bufs=2) as kip,
            tc.tile_pool(name="scr", bufs=2) as scrp,
            tc.tile_pool(name="pp", bufs=2) as psp,
            tc.tile_pool(name="psc", bufs=2, space="PSUM") as psc,
            tc.tile_pool(name="psa", bufs=2, space="PSUM") as psa,
        ):
            # ---- SBUF tiles ----
            w_sb, b_sb = {}, {}
            x_stage = cst.tile([C, SQ], BF16, tag="x_stage")
            y_stage = cst.tile([C, SK], BF16, tag="y_stage")
            x_pad = cst.tile([C, XP * XP], BF16, tag="x_pad")
            y_pad = cst.tile([C, YP * YP], BF16, tag="y_pad")
            t1q = cst.tile([C, XP * XP], BF16, tag="t1q")
            t1k = cst.tile([C, YP * YP], BF16, tag="t1k")
            x_pad3 = x_pad[:].rearrange("p (r c) -> p r c", c=XP)
            y_pad3 = y_pad[:].rearrange("p (r c) -> p r c", c=YP)
            t1q3 = t1q[:].rearrange("p (r c) -> p r c", c=XP)
            t1k3 = t1k[:].rearrange("p (r c) -> p r c", c=YP)
            x_st3 = x_stage[:].rearrange("p (r c) -> p r c", c=A)
            y_st3 = y_stage[:].rearrange("p (r c) -> p r c", c=H)

            q_img = cst.tile([C, SQ], BF16, tag="q_img")
            qT = cst.tile([C, SQ], BF16, tag="qT")
            kT = cst.tile([C, SK], BF16, tag="kT")
            vz_sb = cst.tile([1, SK], BF16, tag="vz_sb")
            v_rep = cst.tile([C, SK], BF16, tag="v_rep")
            negM = cst.tile([C, 8], F32, tag="negM")
            d_all = cst.tile([C, 32], F32, tag="d_all")   # col = c*8 + m
            n_all = cst.tile([C, 32], F32, tag="n_all")

            # ---- DMAs: SP carries q-criticals, ACT the k-side bulk ----
            def load_w(eng, n):
                w_sb[n] = cst.tile([C, 9 * C], BF16, tag=n, name=n + "_sb")
                eng.dma_start(out=w_sb[n][:, 0:576], in_=dw[n].ap()[:, 0:576])
                eng.dma_start(out=w_sb[n][:, 576:1152], in_=dw[n].ap()[:, 576:1152])

            def load_b(eng, n):
                b_sb[n] = cst.tile([C, 1], F32, tag=n, name=n + "_sb")
                eng.dma_start(out=b_sb[n][:], in_=db[n].ap())

            load_w(nc.sync, "wq1"); load_b(nc.sync, "bq1")
            nc.sync.dma_start(out=x_stage[:], in_=dx.ap())
            load_w(nc.sync, "wq2"); load_b(nc.sync, "bq2")
            load_w(nc.scalar, "wk1"); load_b(nc.scalar, "bk1")
            for h in range(2):
                nc.scalar.dma_start(out=y_stage[:, ts(h, SK // 2)],
                                    in_=dy.ap()[:, ts(h, SK // 2)])
            load_w(nc.scalar, "wk2"); load_b(nc.scalar, "bk2")
            nc.scalar.dma_start(out=vz_sb[:], in_=dv.ap())

            # ---- borders + interiors ----
            zrow = cst.tile([C, YP], BF16, tag="zrow")
            nc.vector.memset(zrow[:], 0.0)

            def pad_borders(t3, side):
                zr = zrow[:, 0:side].rearrange("p (a c) -> p a c", a=1)
                zc = zrow[:, 0:side - 2].rearrange("p (r a) -> p r a", a=1)
                nc.vector.tensor_copy(t3[:, 0:1, :], zr)
                nc.vector.tensor_copy(t3[:, side - 1:side, :], zr)
                nc.vector.tensor_copy(t3[:, 1:side - 1, 0:1], zc)
                nc.vector.tensor_copy(t3[:, 1:side - 1, side - 1:side], zc)

            pad_borders(x_pad3, XP)
            pad_borders(t1q3, XP)
            pad_borders(y_pad3, YP)
            pad_borders(t1k3, YP)
            # x interior on DVE (needed first)
            nc.vector.tensor_copy(x_pad3[:, 1:A + 1, 1:A + 1], x_st3)
            # y interior + v broadcast on GpSimd (idle engine), in conv1-k tile order
            for h in range(4):
                nc.gpsimd.tensor_copy(
                    y_pad3[:, 1 + 16 * h: 1 + 16 * (h + 1), 1:H + 1],
                    y_st3[:, ts(h, 16), :])
            for h in range(4):
                nc.gpsimd.partition_broadcast(v_rep[:, ts(h, 1024)],
                                              vz_sb[:, ts(h, 1024)])

            # warm the ScalarE exp table before the attention phase needs it
            dummy = cst.tile([C, 1], F32, tag="dummy")
            nc.scalar.activation(dummy[:], b_sb["bq1"][:], AF.Exp, scale=0.0)

            def conv_mm(ps3, w, i, src3, rows0, nrows, side_c):
                dyy, dxx = i // 3, i % 3
                rhs = src3[:, rows0 + dyy: rows0 + dyy + nrows, dxx: dxx + side_c]
                nc.tensor.matmul(ps3, w[:, ts(i, C)], rhs,
                                 start=(i == 0), stop=(i == 8))

            # ---- q path (acts on ScalarE: idle this early) ----
            for n in range(2):   # conv1: tiles of 16 rows x 32 cols = 512
                ps = psc.tile([C, 512], F32, tag="cps")
                ps3 = ps[:].rearrange("p (r c) -> p r c", c=A)
                for i in range(9):
                    conv_mm(ps3, w_sb["wq1"], i, x_pad3, 16 * n, 16, A)
                nc.scalar.activation(t1q3[:, 16 * n + 1:16 * n + 17, 1:A + 1], ps3,
                                     AF.Relu, bias=b_sb["bq1"][:])
            for n in range(2):   # conv2; bq2 is pre-scaled by s host-side
                ps = psc.tile([C, 512], F32, tag="cps")
                ps3 = ps[:].rearrange("p (r c) -> p r c", c=A)
                for i in range(9):
                    conv_mm(ps3, w_sb["wq2"], i, t1q3, 16 * n, 16, A)
                nc.scalar.activation(q_img[:, ts(n, 512)], ps[:],
                                     AF.Identity, bias=b_sb["bq2"][:], scale=SCALE)
            for g in range(8):   # qT via DMA XBAR transpose (SP)
                nc.sync.dma_start_transpose(out=qT[:, ts(g, C)], in_=q_img[:, ts(g, C)])

            # ---- k path, sliced into single-instruction thunks ----
            def conv1_k_thunks(t):
                box = {}
                th = []

                def mk(i):
                    def f():
                        if i == 0:
                            box['ps'] = psc.tile([C, 512], F32, tag="cps", name="cps")
                        ps3 = box['ps'][:].rearrange("p (r c) -> p r c", c=H)
                        conv_mm(ps3, w_sb["wk1"], i, y_pad3, 8 * t, 8, H)
                    return f
                for i in range(9):
                    th.append(mk(i))

                def act():
                    ps3 = box['ps'][:].rearrange("p (r c) -> p r c", c=H)
                    nc.vector.tensor_scalar(
                        out=t1k3[:, 8 * t + 1:8 * t + 9, 1:H + 1], in0=ps3,
                        scalar1=b_sb["bk1"][:], scalar2=0.0, op0=ALU.add, op1=ALU.max)
                th.append(act)
                return th

            def conv2_k_thunks(t):
                box = {}
                th = []

                def mk(i):
                    def f():
                        if i == 0:
                            box['ps'] = psc.tile([C, 512], F32, tag="cps", name="cps")
                        ps3 = box['ps'][:].rearrange("p (r c) -> p r c", c=H)
                        conv_mm(ps3, w_sb["wk2"], i, t1k3, 8 * t, 8, H)
                    return f
                for i in range(9):
                    th.append(mk(i))

                def act():
                    box['kimg'] = kip.tile([C, 512], BF16, tag="kimg", name="kimg")
                    nc.vector.tensor_scalar(
                        out=box['kimg'][:], in0=box['ps'][:],
                        scalar1=b_sb["bk2"][:], scalar2=None, op0=ALU.add)
                th.append(act)

                def mk_tr(i):
                    def f():
                        nc.sync.dma_start_transpose(
                            out=kT[:, 512 * t + 128 * i: 512 * t + 128 * (i + 1)],
                            in_=box['kimg'][:, ts(i, C)])
                    return f
                for i in range(4):
                    th.append(mk_tr(i))
                return th

            # prologue + chunk-0 pre-work: kT chunk 0 (tiles 0,1).
            # conv2_k(t) needs conv1_k(t) AND conv1_k(t+1) (bottom halo row),
            # so conv1 stays two tiles ahead.
            for f in conv1_k_thunks(0) + conv1_k_thunks(1) \
                    + conv1_k_thunks(2) + conv1_k_thunks(3) \
                    + conv2_k_thunks(0) + conv2_k_thunks(1):
                f()

            def pre_work(cn):
                # work producing kT chunk cn (interleaved into chunk cn-1's m-loop)
                if cn > 3:
                    return []
                th = []
                if 2 * cn + 2 < 8:
                    th += conv1_k_thunks(2 * cn + 2)
                if 2 * cn + 3 < 8:
                    th += conv1_k_thunks(2 * cn + 3)
                th += conv2_k_thunks(2 * cn)
                th += conv2_k_thunks(2 * cn + 1)
                return th

            # ---- attention: 4 chunks of 1024 keys, 8 m-blocks each ----
            pend_stt = None    # deferred numerator STT (breaks Sc->DVE->Sc chain)
            for c in range(4):
                work = pre_work(c + 1)
                wpos = 0
                for m in range(8):
                    psl = psa.tile([C, 1024], F32, tag="psl")
                    for u in range(2):
                        nc.tensor.matmul(psl[:, ts(u, 512)], qT[:, ts(m, C)],
                                         kT[:, 1024 * c + 512 * u: 1024 * c + 512 * (u + 1)],
                                         start=True, stop=True)
                    if c == 0:
                        # global shift: row max over the first 256 chunk-0
                        # columns (two full channel sweeps)
                        nc.vector.tensor_reduce(out=negM[:, m:m + 1], in_=psl[:, 0:256],
                                                axis=mybir.AxisListType.X, op=ALU.max,
                                                negate=True)
                    P = psp.tile([C, 1024], BF16, tag="P")
                    nc.scalar.activation(P[:], psl[:], AF.Exp, bias=negM[:, m:m + 1],
                                         scale=1.0,
                                         accum_out=d_all[:, 8 * c + m: 8 * c + m + 1])
                    # spread next-chunk conv thunks uniformly over the 8 m-iters
                    tgt = ((m + 1) * len(work) + 7) // 8
                    while wpos < min(tgt, len(work)):
                        work[wpos]()
                        wpos += 1
                    if pend_stt is not None:
                        pend_stt()

                    def mk_stt(P=P, c=c, m=m):
                        def f():
                            scrap = scrp.tile([C, 1024], BF16, tag="scrap")
                            nc.vector.scalar_tensor_tensor(
                                out=scrap[:], in0=P[:], scalar=1.0,
                                in1=v_rep[:, ts(c, 1024)],
                                op0=ALU.bypass, op1=ALU.mult,
                                accum_out=n_all[:, 8 * c + m: 8 * c + m + 1])
                        return f
                    pend_stt = mk_stt()
                pend_stt()
                pend_stt = None

            # ---- combine the 4 chunk partials and normalize ----
            den = cst.tile([C, 8], F32, tag="den")
            num = cst.tile([C, 8], F32, tag="num")
            t8 = cst.tile([C, 8], F32, tag="t8")
            recip = cst.tile([C, 8], F32, tag="recip")
            res = cst.tile([C, 8], F32, tag="res")
            nc.vector.tensor_tensor(out=den[:], in0=d_all[:, 0:8], in1=d_all[:, 8:16], op=ALU.add)
            nc.vector.tensor_tensor(out=t8[:], in0=d_all[:, 16:24], in1=d_all[:, 24:32], op=ALU.add)
            nc.vector.tensor_tensor(out=den[:], in0=den[:], in1=t8[:], op=ALU.add)
            nc.vector.tensor_tensor(out=num[:], in0=n_all[:, 0:8], in1=n_all[:, 8:16], op=ALU.add)
            nc.vector.tensor_tensor(out=t8[:], in0=n_all[:, 16:24], in1=n_all[:, 24:32], op=ALU.add)
            nc.vector.tensor_tensor(out=num[:], in0=num[:], in1=t8[:], op=ALU.add)
            nc.vector.reciprocal(recip[:], den[:])
            nc.vector.tensor_tensor(out=res[:], in0=num[:], in1=recip[:], op=ALU.mult)
            nc.sync.dma_start(out=dout.ap().rearrange("(co m) one -> co (m one)", m=8),
                              in_=res[:])

    nc.compile()
    return nc


_NC_CACHE = []


def _build_in_maps(x, y, z, wq1, bq1, wq2, bq2, wk1, bk1, wk2, bk2):
    x = np.asarray(x, dtype=np.float32)
    y = np.asarray(y, dtype=np.float32)
    z = np.asarray(z, dtype=np.float32)
    B = x.shape[0]

    # weights: (co, ci, dy, dx) -> (ci, tap*128+co), bf16
    wmap = {}
    for name, w in (("wq1", wq1), ("wq2", wq2), ("wk1", wk1), ("wk2", wk2)):
        wmap[name] = np.ascontiguousarray(
            np.asarray(w, dtype=np.float32).transpose(1, 2, 3, 0).reshape(C, 9 * C)
        ).astype(ml_dtypes.bfloat16)
    bmap = {"bq1": np.asarray(bq1, np.float32),
            # bq2 pre-scaled: conv2-q act computes s*psum + (s*bq2)
            "bq2": SCALE * np.asarray(bq2, np.float32),
            "bk1": np.asarray(bk1, np.float32),
            "bk2": np.asarray(bk2, np.float32)}
    bmap = {n: np.ascontiguousarray(b.reshape(C, 1)) for n, b in bmap.items()}
    pex = _make_pe(C, SQ).reshape(C, SQ)
    pey = _make_pe(C, SK).reshape(C, SK)
    # v in t-major key order: store[t*128+co] = z_flat[co*32+t]
    zperm = np.ascontiguousarray(
        z.reshape(B, SK).reshape(B, C, SK // C).transpose(0, 2, 1).reshape(B, 1, SK)
    ).astype(ml_dtypes.bfloat16)

    in_maps = []
    for b in range(B):
        m = {
            "x1": (x[b].reshape(C, SQ) + pex).astype(ml_dtypes.bfloat16),
            "y1": (y[b].reshape(C, SK) + pey).astype(ml_dtypes.bfloat16),
            "vz": zperm[b],
        }
        m.update(wmap)
        m.update(bmap)
        in_maps.append(m)
    return in_maps


def kernel(x, y, z, wq1, bq1, wq2, bq2, wk1, bk1, wk2, bk2):
    B = np.asarray(x).shape[0]
    assert B == N_CORES

    if not _NC_CACHE:
        _NC_CACHE.append(_build_program())
    nc = _NC_CACHE[0]

    in_maps = _build_in_maps(x, y, z, wq1, bq1, wq2, bq2, wk1, bk1, wk2, bk2)
    res = run_bass_kernel_spmd(nc, in_maps, core_ids=list(range(N_CORES)))
    out = np.stack([res.results[b]["out"].reshape(SQ, 1) for b in range(B)])
    return out.astype(np.float32)


# revision 9
# speedup vs baseline: 1.3203x; 1.1284x over previous
"""Trainium2 Bass kernel for nn_CrossAttention (8-core data-parallel over batch).

Math (per batch b):
  x1 = x + PEx ; y1 = y + PEy           (raw-reshape positional encodings)
  q  = conv3x3(relu(conv3x3(x1,wq1)+bq1), wq2)+bq2   viewed as (1024,128)
  k  = conv3x3(relu(conv3x3(y1,wk1)+bk1), wk2)+bk2   viewed as (4096,128)
  out = softmax(s * q @ k.T) @ z.flat                (s = 1/sqrt(128))

Device mapping (one batch element per NeuronCore):
  - x+PE folded host-side; all matmul operands BF16 (the only full-rate PE
    dtype on TRN2 silicon: fp32r and fp16 both run as two passes) with fp32
    PSUM accumulation
  - convs as 9 accumulating bf16 matmuls per 512-wide output tile (weights
    stationary (ci,co), padded image moving with 2D shifted APs)
  - q-path conv acts on ScalarE (idle early; SCALE folded into the conv2
    activation's free affine), k-path conv acts on DVE (tensor_scalar with
    per-partition bias AP + relu via the second scalar op)
  - conv outputs transposed into j-major layout on the PE (bf16 transposes
    are single-pass); PSUM->SBUF fixup copies ride on ScalarE
  - v replicated across partitions with a GpSimd partition_broadcast; y-pad
    interior filled by GpSimd copies from a contiguous staging tile; all
    input DMAs issue from the SP HWDGE ring in order of first use
  - softmax shift: ONE per-row shift from chunk 0's first-256-column
    subsample (worst observed full-row gap is 36 << 88, so exp stays in
    fp32/bf16 range and no online rescaling is needed; partial num/den sums
    accumulate additively across the 4 key-chunks)
  - exp on ScalarE with fused per-partition bias + accumulated denominator
    (the compiler forbids non-matmul BF16 PSUM writes, so P lands in SBUF);
    numerator via DVE
    scalar_tensor_tensor (P*v with fused sum), deferred one m-iteration so
    ScalarE->DVE->ScalarE serialization never forms
  - k-path conv work is sliced into single-instruction thunks and spread
    uniformly across the attention m-iterations so the PE stays dense (and
    HAM-warm) while ScalarE chews the exps
"""

import numpy as np
import ml_dtypes

import concourse.bass as bass
import concourse.mybir as mybir
import concourse.tile as tile
from concourse import bacc
from concourse.bass import ts
from concourse.bass_utils import run_bass_kernel_spmd

F32 = mybir.dt.float32
BF16 = mybir.dt.bfloat16
AF = mybir.ActivationFunctionType
ALU = mybir.AluOpType

C = 128
A = 32          # q spatial side
H = 64          # k spatial side
SQ = A * A      # 1024
SK = H * H      # 4096
SCALE = float(C ** -0.5)
N_CORES = 8


def _make_pe(dim, length):
    pos = np.arange(length, dtype=np.float32)[:, None]
    div = np.exp(np.arange(0, dim, 2, dtype=np.float32) * np.float32(-np.log(10000.0) / dim))
    pe = np.zeros((length, dim), dtype=np.float32)
    pe[:, 0::2] = np.sin(pos * div)
    pe[:, 1::2] = np.cos(pos * div)
    return pe


def _build_program():
    nc = bacc.Bacc("TRN2", target_bir_lowering=False, debug=False, num_devices=N_CORES)

    dx = nc.dram_tensor("x1", [C, SQ], BF16, kind="ExternalInput")
    dy = nc.dram_tensor("y1", [C, SK], BF16, kind="ExternalInput")
    dv = nc.dram_tensor("vz", [1, SK], BF16, kind="ExternalInput")
    dw = {n: nc.dram_tensor(n, [C, 9 * C], BF16, kind="ExternalInput")
          for n in ("wq1", "wq2", "wk1", "wk2")}
    db = {n: nc.dram_tensor(n, [C, 1], F32, kind="ExternalInput")
          for n in ("bq1", "bq2", "bk1", "bk2")}
    dident = nc.dram_tensor("ident", [C, C], BF16, kind="ExternalInput")
    dout = nc.dram_tensor("out", [SQ, 1], F32, kind="ExternalOutput")

    XP, YP = A + 2, H + 2          # padded sides: 34, 66
    with tile.TileContext(nc) as tc:
        with (
            tc.tile_pool(name="const", bufs=1) as cst,
            tc.tile_pool(name="kimg", bufs=2) as kip,
            tc.tile_pool(name="scr", bufs=2) as scrp,
            tc.tile_pool(name="pp", bufs=2) as psp,
            tc.tile_pool(name="psc", bufs=2, space="PSUM") as psc,
            tc.tile_pool(name="psa", bufs=2, space="PSUM") as psa,
            tc.tile_pool(name="pst", bufs=2, space="PSUM") as pst,
        ):
            # ---- SBUF tiles ----
            w_sb, b_sb = {}, {}
            x_stage = cst.tile([C, SQ], BF16, tag="x_stage")
            y_stage = cst.tile([C, SK], BF16, tag="y_stage")
            x_pad = cst.tile([C, XP * XP], BF16, tag="x_pad")
            y_pad = cst.tile([C, YP * YP], BF16, tag="y_pad")
            t1q = cst.tile([C, XP * XP], BF16, tag="t1q")
            t1k = cst.tile([C, YP * YP], BF16, tag="t1k")
            x_pad3 = x_pad[:].rearrange("p (r c) -> p r c", c=XP)
            y_pad3 = y_pad[:].rearrange("p (r c) -> p r c", c=YP)
            t1q3 = t1q[:].rearrange("p (r c) -> p r c", c=XP)
            t1k3 = t1k[:].rearrange("p (r c) -> p r c", c=YP)
            x_st3 = x_stage[:].rearrange("p (r c) -> p r c", c=A)
            y_st3 = y_stage[:].rearrange("p (r c) -> p r c", c=H)

            q_img = cst.tile([C, SQ], BF16, tag="q_img")
            qT = cst.tile([C, SQ], BF16, tag="qT")
            kT = cst.tile([C, SK], BF16, tag="kT")
            vz_sb = cst.tile([1, SK], BF16, tag="vz_sb")
            v_rep = cst.tile([C, SK], BF16, tag="v_rep")
            negM = cst.tile([C, 8], F32, tag="negM")
            d_all = cst.tile([C, 32], F32, tag="d_all")   # col = c*8 + m
            n_all = cst.tile([C, 32], F32, tag="n_all")

            # ---- DMAs: SP carries q-criticals, ACT the k-side bulk ----
            def load_w(eng, n):
                w_sb[n] = cst.tile([C, 9 * C], BF16, tag=n, name=n + "_sb")
                eng.dma_start(out=w_sb[n][:, 0:576], in_=dw[n].ap()[:, 0:576])
                eng.dma_start(out=w_sb[n][:, 576:1152], in_=dw[n].ap()[:, 576:1152])

            def load_b(eng, n):
                b_sb[n] = cst.tile([C, 1], F32, tag=n, name=n + "_sb")
                eng.dma_start(out=b_sb[n][:], in_=db[n].ap())

            ident = cst.tile([C, C], BF16, tag="ident")
            load_w(nc.sync, "wq1"); load_b(nc.sync, "bq1")
            nc.sync.dma_start(out=x_stage[:], in_=dx.ap())
            load_w(nc.sync, "wq2"); load_b(nc.sync, "bq2")
            nc.sync.dma_start(out=ident[:], in_=dident.ap())
            load_w(nc.sync, "wk1"); load_b(nc.sync, "bk1")
            for h in range(2):
                nc.sync.dma_start(out=y_stage[:, ts(h, SK // 2)],
                                  in_=dy.ap()[:, ts(h, SK // 2)])
            load_w(nc.sync, "wk2"); load_b(nc.sync, "bk2")
            nc.sync.dma_start(out=vz_sb[:], in_=dv.ap())

            # ---- borders + interiors ----
            zrow = cst.tile([C, YP], BF16, tag="zrow")
            nc.vector.memset(zrow[:], 0.0)

            def pad_borders(t3, side):
                zr = zrow[:, 0:side].rearrange("p (a c) -> p a c", a=1)
                zc = zrow[:, 0:side - 2].rearrange("p (r a) -> p r a", a=1)
                nc.vector.tensor_copy(t3[:, 0:1, :], zr)
                nc.vector.tensor_copy(t3[:, side - 1:side, :], zr)
                nc.vector.tensor_copy(t3[:, 1:side - 1, 0:1], zc)
                nc.vector.tensor_copy(t3[:, 1:side - 1, side - 1:side], zc)

            pad_borders(x_pad3, XP)
            pad_borders(t1q3, XP)
            pad_borders(y_pad3, YP)
            pad_borders(t1k3, YP)
            # x interior on DVE (needed first)
            nc.vector.tensor_copy(x_pad3[:, 1:A + 1, 1:A + 1], x_st3)
            # y interior + v broadcast on GpSimd (idle engine), in need order
            for h in range(2):
                nc.gpsimd.tensor_copy(
                    y_pad3[:, 1 + 16 * h: 1 + 16 * (h + 1), 1:H + 1],
                    y_st3[:, ts(h, 16), :])
            nc.gpsimd.partition_broadcast(v_rep[:, ts(0, 1024)], vz_sb[:, ts(0, 1024)])
            for h in range(2, 4):
                nc.gpsimd.tensor_copy(
                    y_pad3[:, 1 + 16 * h: 1 + 16 * (h + 1), 1:H + 1],
                    y_st3[:, ts(h, 16), :])
            for h in range(1, 4):
                nc.gpsimd.partition_broadcast(v_rep[:, ts(h, 1024)],
                                              vz_sb[:, ts(h, 1024)])

            # warm the ScalarE exp table before the attention phase needs it
            dummy = cst.tile([C, 1], F32, tag="dummy")
            nc.scalar.activation(dummy[:], b_sb["bq1"][:], AF.Exp, scale=0.0)

            def conv_mm(ps3, w, i, src3, rows0, nrows, side_c):
                dyy, dxx = i // 3, i % 3
                rhs = src3[:, rows0 + dyy: rows0 + dyy + nrows, dxx: dxx + side_c]
                nc.tensor.matmul(ps3, w[:, ts(i, C)], rhs,
                                 start=(i == 0), stop=(i == 8))

            # ---- q path (acts on ScalarE: idle this early) ----
            for n in range(2):   # conv1: tiles of 16 rows x 32 cols = 512
                ps = psc.tile([C, 512], F32, tag="cps")
                ps3 = ps[:].rearrange("p (r c) -> p r c", c=A)
                for i in range(9):
                    conv_mm(ps3, w_sb["wq1"], i, x_pad3, 16 * n, 16, A)
                nc.scalar.activation(t1q3[:, 16 * n + 1:16 * n + 17, 1:A + 1], ps3,
                                     AF.Relu, bias=b_sb["bq1"][:])
            for n in range(2):   # conv2; bq2 is pre-scaled by s host-side
                ps = psc.tile([C, 512], F32, tag="cps")
                ps3 = ps[:].rearrange("p (r c) -> p r c", c=A)
                for i in range(9):
                    conv_mm(ps3, w_sb["wq2"], i, t1q3, 16 * n, 16, A)
                nc.scalar.activation(q_img[:, ts(n, 512)], ps[:],
                                     AF.Identity, bias=b_sb["bq2"][:], scale=SCALE)
            for g in range(2):   # qT via PE transposes; ScalarE PSUM->SBUF fixup
                pt = pst.tile([C, 512], BF16, tag="tps")
                for i in range(4):
                    nc.tensor.transpose(pt[:, ts(i, C)],
                                        q_img[:, ts(4 * g + i, C)], ident[:])
                nc.scalar.activation(qT[:, ts(g, 512)], pt[:], AF.Identity)

            # ---- k path, sliced into single-instruction thunks ----
            def conv1_k_thunks(t):
                box = {}
                th = []

                def mk(i):
                    def f():
                        if i == 0:
                            box['ps'] = psc.tile([C, 512], F32, tag="cps", name="cps")
                        ps3 = box['ps'][:].rearrange("p (r c) -> p r c", c=H)
                        conv_mm(ps3, w_sb["wk1"], i, y_pad3, 8 * t, 8, H)
                    return f
                for i in range(9):
                    th.append(mk(i))

                def act():
                    ps3 = box['ps'][:].rearrange("p (r c) -> p r c", c=H)
                    nc.vector.tensor_scalar(
                        out=t1k3[:, 8 * t + 1:8 * t + 9, 1:H + 1], in0=ps3,
                        scalar1=b_sb["bk1"][:], scalar2=0.0, op0=ALU.add, op1=ALU.max)
                th.append(act)
                return th

            def conv2_k_thunks(t):
                box = {}
                th = []

                def mk(i):
                    def f():
                        if i == 0:
                            box['ps'] = psc.tile([C, 512], F32, tag="cps", name="cps")
                        ps3 = box['ps'][:].rearrange("p (r c) -> p r c", c=H)
                        conv_mm(ps3, w_sb["wk2"], i, t1k3, 8 * t, 8, H)
                    return f
                for i in range(9):
                    th.append(mk(i))

                def act():
                    box['kimg'] = kip.tile([C, 512], BF16, tag="kimg", name="kimg")
                    nc.vector.tensor_scalar(
                        out=box['kimg'][:], in0=box['ps'][:],
                        scalar1=b_sb["bk2"][:], scalar2=None, op0=ALU.add)
                th.append(act)

                def mk_tr(i):
                    def f():
                        if i == 0:
                            box['pt'] = pst.tile([C, 512], BF16, tag="tps", name="tps")
                        nc.tensor.transpose(box['pt'][:, ts(i, C)],
                                            box['kimg'][:, ts(i, C)], ident[:])
                    return f
                for i in range(4):
                    th.append(mk_tr(i))

                def cp():
                    nc.scalar.activation(kT[:, ts(t, 512)], box['pt'][:], AF.Identity)
                th.append(cp)
                return th

            # prologue + chunk-0 pre-work: kT chunk 0 (tiles 0,1).
            # conv2_k(t) needs conv1_k(t) AND conv1_k(t+1) (bottom halo row),
            # so conv1 stays two tiles ahead.
            for f in conv1_k_thunks(0) + conv1_k_thunks(1) \
                    + conv1_k_thunks(2) + conv1_k_thunks(3) \
                    + conv2_k_thunks(0) + conv2_k_thunks(1):
                f()

            def pre_work(cn):
                # work producing kT chunk cn (interleaved into chunk cn-1's m-loop)
                if cn > 3:
                    return []
                th = []
                if 2 * cn + 2 < 8:
                    th += conv1_k_thunks(2 * cn + 2)
                if 2 * cn + 3 < 8:
                    th += conv1_k_thunks(2 * cn + 3)
                th += conv2_k_thunks(2 * cn)
                th += conv2_k_thunks(2 * cn + 1)
                return th

            # ---- attention: 4 chunks of 1024 keys, 8 m-blocks each ----
            pend_stt = None    # deferred numerator STT (breaks Sc->DVE->Sc chain)
            for c in range(4):
                work = pre_work(c + 1)
                wpos = 0
                for m in range(8):
                    psl = psa.tile([C, 1024], F32, tag="psl")
                    for u in range(2):
                        nc.tensor.matmul(psl[:, ts(u, 512)], qT[:, ts(m, C)],
                                         kT[:, 1024 * c + 512 * u: 1024 * c + 512 * (u + 1)],
                                         start=True, stop=True)
                    if c == 0:
                        # global shift: row max over the first 256 chunk-0
                        # columns (two full channel sweeps)
                        nc.vector.tensor_reduce(out=negM[:, m:m + 1], in_=psl[:, 0:256],
                                                axis=mybir.AxisListType.X, op=ALU.max,
                                                negate=True)
                    P = psp.tile([C, 1024], BF16, tag="P")
                    nc.scalar.activation(P[:], psl[:], AF.Exp, bias=negM[:, m:m + 1],
                                         scale=1.0,
                                         accum_out=d_all[:, 8 * c + m: 8 * c + m + 1])
                    # spread next-chunk conv thunks uniformly over the 8 m-iters
                    tgt = ((m + 1) * len(work) + 7) // 8
                    while wpos < min(tgt, len(work)):
                        work[wpos]()
                        wpos += 1
                    if pend_stt is not None:
                        pend_stt()

                    def mk_stt(P=P, c=c, m=m):
                        def f():
                            scrap = scrp.tile([C, 1024], BF16, tag="scrap")
                            nc.vector.scalar_tensor_tensor(
                                out=scrap[:], in0=P[:], scalar=1.0,
                                in1=v_rep[:, ts(c, 1024)],
                                op0=ALU.bypass, op1=ALU.mult,
                                accum_out=n_all[:, 8 * c + m: 8 * c + m + 1])
                        return f
                    pend_stt = mk_stt()
                pend_stt()
                pend_stt = None

            # ---- combine the 4 chunk partials and normalize ----
            den = cst.tile([C, 8], F32, tag="den")
            num = cst.tile([C, 8], F32, tag="num")
            t8 = cst.tile([C, 8], F32, tag="t8")
            recip = cst.tile([C, 8], F32, tag="recip")
            res = cst.tile([C, 8], F32, tag="res")
            nc.vector.tensor_tensor(out=den[:], in0=d_all[:, 0:8], in1=d_all[:, 8:16], op=ALU.add)
            nc.vector.tensor_tensor(out=t8[:], in0=d_all[:, 16:24], in1=d_all[:, 24:32], op=ALU.add)
            nc.vector.tensor_tensor(out=den[:], in0=den[:], in1=t8[:], op=ALU.add)
            nc.vector.tensor_tensor(out=num[:], in0=n_all[:, 0:8], in1=n_all[:, 8:16], op=ALU.add)
            nc.vector.tensor_tensor(out=t8[:], in0=n_all[:, 16:24], in1=n_all[:, 24:32], op=ALU.add)
            nc.vector.tensor_tensor(out=num[:], in0=num[:], in1=t8[:], op=ALU.add)
            nc.vector.reciprocal(recip[:], den[:])
            nc.vector.tensor_tensor(out=res[:], in0=num[:], in1=recip[:], op=ALU.mult)
            nc.sync.dma_start(out=dout.ap().rearrange("(co m) one -> co (m one)", m=8),
                              in_=res[:])

    nc.compile()
    return nc


_NC_CACHE = []


def _build_in_maps(x, y, z, wq1, bq1, wq2, bq2, wk1, bk1, wk2, bk2):
    x = np.asarray(x, dtype=np.float32)
    y = np.asarray(y, dtype=np.float32)
    z = np.asarray(z, dtype=np.float32)
    B = x.shape[0]

    # weights: (co, ci, dy, dx) -> (ci, tap*128+co), bf16
    wmap = {}
    for name, w in (("wq1", wq1), ("wq2", wq2), ("wk1", wk1), ("wk2", wk2)):
        wmap[name] = np.ascontiguousarray(
            np.asarray(w, dtype=np.float32).transpose(1, 2, 3, 0).reshape(C, 9 * C)
        ).astype(ml_dtypes.bfloat16)
    bmap = {"bq1": np.asarray(bq1, np.float32),
            # bq2 pre-scaled: conv2-q act computes s*psum + (s*bq2)
            "bq2": SCALE * np.asarray(bq2, np.float32),
            "bk1": np.asarray(bk1, np.float32),
            "bk2": np.asarray(bk2, np.float32)}
    bmap = {n: np.ascontiguousarray(b.reshape(C, 1)) for n, b in bmap.items()}
    pex = _make_pe(C, SQ).reshape(C, SQ)
    pey = _make_pe(C, SK).reshape(C, SK)
    # v in t-major key order: store[t*128+co] = z_flat[co*32+t]
    zperm = np.ascontiguousarray(
        z.reshape(B, SK).reshape(B, C, SK // C).transpose(0, 2, 1).reshape(B, 1, SK)
    ).astype(ml_dtypes.bfloat16)

    in_maps = []
    for b in range(B):
        m = {
            "x1": (x[b].reshape(C, SQ) + pex).astype(ml_dtypes.bfloat16),
            "y1": (y[b].reshape(C, SK) + pey).astype(ml_dtypes.bfloat16),
            "vz": zperm[b],
            "ident": np.eye(C, dtype=ml_dtypes.bfloat16),
        }
        m.update(wmap)
        m.update(bmap)
        in_maps.append(m)
    return in_maps


def kernel(x, y, z, wq1, bq1, wq2, bq2, wk1, bk1, wk2, bk2):
    B = np.asarray(x).shape[0]
    assert B == N_CORES

    if not _NC_CACHE:
        _NC_CACHE.append(_build_program())
    nc = _NC_CACHE[0]

    in_maps = _build_in_maps(x, y, z, wq1, bq1, wq2, bq2, wk1, bk1, wk2, bk2)
    res = run_bass_kernel_spmd(nc, in_maps, core_ids=list(range(N_CORES)))
    out = np.stack([res.results[b]["out"].reshape(SQ, 1) for b in range(B)])
    return out.astype(np.float32)
